# revision 29
# baseline (speedup 1.0000x reference)
"""Trainium2 Bass kernel for nn_CrossAttentionFusion.

Problem (hardcoded shapes): B=2, C1=64, C2=256, D=256, NH=8, HD=32, H=W=64,
n = H*W = 4096 tokens per batch image.

    xl = F_lidar tokens (B, n, C1); xc = F_cam tokens (B, n, C2)
    Q = xl@Wq^T, K = xc@Wk^T, V = xc@Wv^T  (per-head HD=32)
    attn = softmax(QK^T/sqrt(HD)); out = attn@V
    x = LN1(xl@Wres^T + out@Wo^T); x = LN2(x + FFN(x)); return (B, D, H, W)

Sharding: 8 cores, zero collectives. Core i handles batch b=i//4 and the
1024-token q-slice (i%4). K/V for the whole image are recomputed per core.

v3 changes (vs the 343us v2):
  * Startup: input DMAs fan out over four engine queues (sync/tensor/
    vector/gpsimd) instead of serializing ~22us on the sync queue.
  * One ACT table set for the whole kernel (natural_log_exp_and_others):
    LN rstd is exp(-0.5*ln(var+eps)) instead of Sqrt + DVE reciprocal, so
    no mid-kernel ACT_TABLE_LOAD thrash (6 x 1.3us on the v2 trace).
  * resid = xl@Wres is folded into the Wo PSUM accumulation in phase C
    (3 accumulating matmuls per q-tile); the separate phase-A resid pass,
    its SBUF buffer, and its DVE copies are gone.
  * Phase C/D rebalanced: LN apply and transpose-copyouts alternate
    ScalarE(Identity scale/bias per-partition)/DVE, LN1 affine folded into
    the transpose copy-out (g1/b1 are per-partition in the transposed
    layout), FFN1 relu+bias alternates ScalarE/DVE, bf2 enters FFN2 as a
    1-row ones matmul, LN2 residual (x1) re-enters via identity-rhs
    matmuls accumulating into the FFN2 PSUM, final g2/b2 affine split
    DVE/GpSimd.
  * KT/QT/V PSUM->SBUF copy-outs alternate ScalarE/DVE.

Attention core (unchanged from v2): bf16 QK^T with 2-head row packing;
exp split ScalarE Exp (qc=0) / DVE Schraudolph int16 (qc=1); AV + ones-
denominator column-packed 4-wide; 3-deep PSUM score rotation.
"""

import numpy as np

B, C1, C2, D, NH, H, W = 2, 64, 256, 256, 8, 64, 64
HD = D // NH                 # 32
N_TOK = H * W                # 4096 tokens per image
N_CORES = 8
CORES_PER_B = N_CORES // B   # 4
NQ = N_TOK // CORES_PER_B    # 1024 q tokens per core
EPS = 1e-5
SCALE = HD ** -0.5
KC = N_TOK // 128            # 32 k-chunks
QT_TILES = NQ // 128         # 8 q-tiles of 128
F1 = 4 * D                   # 1024 FFN hidden

# Schraudolph-style exp for bf16-bit-pattern in int16 (trunc semantics):
#   bf16_bits(exp(s*SCALE)) ~= int16(EXP_A*s + EXP_B)
EXP_A = 184.6649652337873 * SCALE
EXP_B = 16250.89

_built = None


def _build():
    from contextlib import ExitStack

    import concourse.mybir as mybir
    import concourse.tile as tile
    from concourse import bacc
    from concourse.masks import make_identity

    F32 = mybir.dt.float32
    BF16 = mybir.dt.bfloat16
    I16 = mybir.dt.int16
    I8 = mybir.dt.int8
    FP8 = mybir.dt.float8e4
    FP8E5 = mybir.dt.float8e5
    AF = mybir.ActivationFunctionType
    OP = mybir.AluOpType

    import bass_rust as _bass_rust
    from concourse.hw_specs import get_activation_tables

    class OneSetBacc(bacc.Bacc):
        # Force every ACT table load to natural_log_exp_and_others (it
        # contains exp/ln/identity/copy/relu — everything this kernel
        # uses), so the kernel pays exactly one ACT_TABLE_LOAD instead
        # of thrashing between per-function home sets.
        def insert_act_table_loads(self):
            has_activation = any(
                isinstance(i, mybir.InstActivation)
                for b in self.main_func.blocks
                for i in b.instructions
            )
            if not has_activation:
                return
            keep = "natural_log_exp_and_others"
            tables = [(name, (fns if name == keep else set()))
                      for name, fns in
                      get_activation_tables(self.m.arch).items()]
            _bass_rust.insert_act_table_loads(self, tables)

    nc = OneSetBacc(trn_type="TRN2", target_bir_lowering=False, debug=False,
                    num_devices=N_CORES)

    # ---- DRAM I/O ----
    xq = nc.dram_tensor("xq", [C1, NQ], BF16, kind="ExternalInput").ap()
    xqf = nc.dram_tensor("xqf", [C1, NQ], F32, kind="ExternalInput").ap()
    xc = nc.dram_tensor("xc", [C2, N_TOK], BF16, kind="ExternalInput").ap()
    wkt = nc.dram_tensor("wkt", [C2, D], BF16, kind="ExternalInput").ap()
    wvt = nc.dram_tensor("wvt", [C2, D], BF16, kind="ExternalInput").ap()
    wqt = nc.dram_tensor("wqt", [C1, D], BF16, kind="ExternalInput").ap()
    wrt = nc.dram_tensor("wrt", [C1, D], F32, kind="ExternalInput").ap()
    wot = nc.dram_tensor("wot", [D, D], BF16, kind="ExternalInput").ap()
    w1t = nc.dram_tensor("w1t", [D, F1], BF16, kind="ExternalInput").ap()
    w2t = nc.dram_tensor("w2t", [F1, D], BF16, kind="ExternalInput").ap()
    g1 = nc.dram_tensor("g1", [D], F32, kind="ExternalInput").ap()
    b1 = nc.dram_tensor("b1", [D], F32, kind="ExternalInput").ap()
    g2 = nc.dram_tensor("g2", [D], F32, kind="ExternalInput").ap()
    b2 = nc.dram_tensor("b2", [D], F32, kind="ExternalInput").ap()
    bf1 = nc.dram_tensor("bf1", [F1], F32, kind="ExternalInput").ap()
    bf2 = nc.dram_tensor("bf2", [D], BF16, kind="ExternalInput").ap()
    out = nc.dram_tensor("out", [NQ, D], F32, kind="ExternalOutput").ap()

    with tile.TileContext(nc) as tc, ExitStack() as ctx:
        # ---- persistent SBUF ----
        P = ctx.enter_context(tc.tile_pool(name="persist", bufs=1))

        xq_sb = P.tile([C1, NQ], BF16, name="xq_sb")
        wkt_sb = [P.tile([128, D], BF16, name=f"wkt{c}") for c in range(2)]
        wvt_sb = [P.tile([128, D], BF16, name=f"wvt{c}") for c in range(2)]
        wqt_sb = P.tile([C1, D], BF16, name="wqt_sb")
        wrt_sb = P.tile([128, D], F32, name="wrt_sb")
        xqf_sb = P.tile([128, NQ], F32, name="xqf_sb")
        wot_sb = [P.tile([128, D], BF16, name=f"wot{c}") for c in range(2)]
        kt_sb = [P.tile([128, N_TOK], BF16, name=f"kt{g}") for g in range(2)]
        v_sb = P.tile([128, KC, D], BF16, name="v_sb")
        qt_sb = [P.tile([128, NQ], BF16, name=f"qt{g}") for g in range(2)]
        attn_sb = [P.tile([128, NQ], BF16, name=f"attn{g}") for g in range(2)]
        rec_bc = [P.tile([128, NQ], F32, name=f"recbc{g}") for g in range(2)]
        ones_bf = P.tile([128, HD], BF16, name="ones_bf")
        ident = P.tile([128, 128], F32, name="ident")
        i2_bf = P.tile([128, 512], BF16, name="i2_bf")
        eps_sb = P.tile([128, 1], F32, name="eps_sb")
        g1_col = P.tile([128, 2], F32, name="g1_col")
        b1_col = P.tile([128, 2], F32, name="b1_col")
        g2_bc = P.tile([128, D], F32, name="g2_bc")
        b2_bc = P.tile([128, D], F32, name="b2_bc")
        bf1_col = P.tile([128, 8], F32, name="bf1_col")
        ones_row = P.tile([1, 128], BF16, name="ones_row")
        bf2_row = P.tile([1, D], BF16, name="bf2_row")

        ones_f32 = P.tile([128, HD], F32, name="ones_f32")
        nc.vector.memset(ones_f32, 1.0)
        nc.vector.memset(xqf_sb[C1:128, :], 0.0)
        nc.vector.memset(wrt_sb[C1:128, :], 0.0)
        nc.vector.tensor_copy(ones_bf, ones_f32)
        nc.vector.memset(ones_row, 1.0)
        nc.vector.memset(eps_sb, EPS)
        make_identity(nc, ident)
        nc.vector.memset(i2_bf, 0.0)
        nc.vector.tensor_copy(i2_bf[:, 0:128], ident)
        nc.vector.tensor_copy(i2_bf[:, 384:512], ident)

        def bcast_row(dst, src_ap, eng=None):
            # (n,) dram -> (128, n) sbuf, replicated on all partitions
            import concourse.bass as bass
            src = bass.AP(tensor=src_ap.tensor, offset=src_ap.offset,
                          ap=[[0, 128]] + src_ap.ap)
            (eng or nc.sync).dma_start(dst, src)

        # --- startup DMAs fanned out across the three DMA-capable
        # queues (sync/scalar/gpsimd) instead of serializing on sync ---
        # sync: the K-projection critical path (wkt, then xc half 0).
        for c in range(2):
            nc.sync.dma_start(wkt_sb[c], wkt[128 * c:128 * (c + 1), :])
        # scalar queue: Q-side + xc half 1 + V/resid weights (all done
        # well before ScalarE's first PSUM copy-out).
        nc.scalar.dma_start(wqt_sb, wqt)
        nc.scalar.dma_start(xq_sb, xq)
        # gpsimd queue: LN constants + phase C/D weights.
        nc.gpsimd.dma_start(g1_col, g1.rearrange("(a p) -> p a", p=128))
        nc.gpsimd.dma_start(b1_col, b1.rearrange("(a p) -> p a", p=128))
        bcast_row(g2_bc, g2, nc.gpsimd)
        bcast_row(b2_bc, b2, nc.gpsimd)
        nc.gpsimd.dma_start(bf1_col, bf1.rearrange("(a p) -> p a", p=128))
        nc.gpsimd.dma_start(bf2_row, bf2.rearrange("(o d) -> o d", o=1))

        # =============== Phase A: projections (bf16) ===============
        # KT/QT first in their own PSUM pool. V then runs THROUGH the
        # attention score-buffer rotation, so the attention prologue
        # (first scores + exps) interleaves mid-V and the PE never sees
        # an idle window at the phase transition.
        with tc.tile_pool(name="xc_pool", bufs=1) as XP:
            xc_sb = [XP.tile([128, N_TOK], BF16, name=f"xc{c}")
                     for c in range(2)]
            for ch in range(4):
                cs = slice(1024 * ch, 1024 * (ch + 1))
                nc.sync.dma_start(xc_sb[0][:, cs], xc[0:128, cs])
                nc.scalar.dma_start(xc_sb[1][:, cs], xc[128:256, cs])
            for c in range(2):
                nc.scalar.dma_start(wvt_sb[c], wvt[128 * c:128 * (c + 1), :])
            nc.scalar.dma_start(xqf_sb[0:C1, :], xqf)
            nc.scalar.dma_start(wrt_sb[0:C1, :], wrt)
            for c in range(2):
                nc.scalar.dma_start(wot_sb[c], wot[128 * c:128 * (c + 1), :])
            # phase D weights on the gpsimd queue, behind the LN consts.
            w1t_sb = [P.tile([128, F1], BF16, name=f"w1t{c}")
                      for c in range(2)]
            for c in range(2):
                nc.gpsimd.dma_start(w1t_sb[c], w1t[128 * c:128 * (c + 1), :])
            w2t_sb = P.tile([128, 8, D], BF16, name="w2t_sb")
            nc.gpsimd.dma_start(
                w2t_sb, w2t.rearrange("(a p) d -> p a d", p=128))

            with tc.tile_pool(name="psA", bufs=4, space="PSUM") as psA:
                # KT[d,k] = sum_c WkT[c,d] * xcT[c,k]
                for g in range(2):
                    for ks in range(8):
                        kp = psA.tile([128, 512], F32, name="kp")
                        for c in range(2):
                            nc.tensor.matmul(
                                kp, wkt_sb[c][:, 128 * g:128 * (g + 1)],
                                xc_sb[c][:, 512 * ks:512 * (ks + 1)],
                                start=(c == 0), stop=(c == 1))
                        dst = kt_sb[g][:, 512 * ks:512 * (ks + 1)]
                        if ks % 2 == 0:
                            nc.scalar.copy(dst, kp)
                        else:
                            nc.vector.tensor_copy(dst, kp)
                # QT[d,q] = sum_c WqT[c,d] * xqT[c,q]
                for g in range(2):
                    for qs in range(NQ // 512):
                        qp = psA.tile([128, 512], F32, name="kp")
                        nc.tensor.matmul(
                            qp, wqt_sb[:, 128 * g:128 * (g + 1)],
                            xq_sb[:, 512 * qs:512 * (qs + 1)],
                            start=True, stop=True)
                        dst = qt_sb[g][:, 512 * qs:512 * (qs + 1)]
                        if qs % 2 == 0:
                            nc.scalar.copy(dst, qp)
                        else:
                            nc.vector.tensor_copy(dst, qp)

            # ===== Phase B: V tail + attention =====
            # Attention runs in two q-half passes. Per (head-quad, kc)
            # unit one span of FOUR row-packed score matmuls (rows
            # 0/32/64/96, N=512) writes two [128,1024] PSUM tiles -- four
            # distinct banks, so the concurrent writes are legal. The e
            # tiles keep the baseline [hA 512q | hB 512q] layout, so AV +
            # ones-denominator reuse the proven column-packed 4-wide
            # structure on [128,512] accumulators (1 bank per head-pair).
            # exp alternates ScalarE Exp / DVE int16 Schraudolph per tile.
            with tc.tile_pool(name="scps", bufs=3, space="PSUM") as scps, \
                 tc.tile_pool(name="avps", bufs=1, space="PSUM") as avps, \
                 tc.tile_pool(name="epool", bufs=12) as epool:
                scs, es, av_of = {}, {}, {}
                exp_n = [0]

                def emit_scores(qc, hq, kc):
                    ks = slice(128 * kc, 128 * (kc + 1))
                    qs = slice(512 * qc, 512 * (qc + 1))
                    tiles = []
                    for a in range(2):
                        sc = scps.tile([128, 1024], F32, name="sc")
                        scs[(qc, hq, kc, a)] = sc
                        tiles.append(sc)
                    for m in range(4):
                        nc.tensor.matmul(
                            tiles[m // 2][:, 512 * (m % 2):512 * (m % 2 + 1)],
                            kt_sb[hq][32 * m:32 * m + 32, ks],
                            qt_sb[hq][32 * m:32 * m + 32, qs],
                            start=True, stop=True,
                            tile_position=(32 * m, 0))

                def emit_exps(qc, hq, kc):
                    # ScalarE is slightly faster per tile and carries less
                    # side work, so it takes BOTH tiles every 16th unit
                    # (17:15 split) to keep the two engines in lockstep.
                    exp_n[0] += 1
                    both_s = (exp_n[0] % 16 == 0)
                    for a in range(2):
                        sc = scs.pop((qc, hq, kc, a))
                        e = epool.tile([128, 1024], BF16, name="e")
                        es[(qc, hq, kc, a)] = e
                        if a == 0 or both_s:
                            nc.scalar.activation(e, sc, AF.Exp, scale=SCALE)
                        else:
                            nc.vector.tensor_scalar(
                                out=e.bitcast(I16), in0=sc,
                                scalar1=EXP_A, scalar2=EXP_B,
                                op0=OP.mult, op1=OP.add)

                def emit_avs(qc, hq, kc):
                    if kc == 0:
                        av_of[hq] = [
                            avps.tile([128, 512], F32, name=f"av{a}")
                            for a in range(2)]
                    st, sp = (kc == 0), (kc == KC - 1)
                    for a in range(2):
                        av = av_of[hq][a]
                        hA, hB = 4 * hq + 2 * a, 4 * hq + 2 * a + 1
                        pA, pB = 64 * a, 64 * a + 32
                        oA, oB = (64 * a + 64) % 128, (64 * a + 96) % 128
                        e = es.pop((qc, hq, kc, a))
                        nc.tensor.matmul(
                            av[pA:pA + 32, :],
                            v_sb[:, kc, HD * hA:HD * hA + HD],
                            e[:, 0:512], start=st, stop=sp,
                            tile_position=(0, pA), skip_group_check=True)
                        nc.tensor.matmul(
                            av[pB:pB + 32, :],
                            v_sb[:, kc, HD * hB:HD * hB + HD],
                            e[:, 512:1024], start=st, stop=sp,
                            tile_position=(0, pB), skip_group_check=True)
                        nc.tensor.matmul(
                            av[oA:oA + 32, :], ones_bf, e[:, 0:512],
                            start=st, stop=sp, tile_position=(0, oA),
                            skip_group_check=True)
                        nc.tensor.matmul(
                            av[oB:oB + 32, :], ones_bf, e[:, 512:1024],
                            start=st, stop=sp, tile_position=(0, oB),
                            skip_group_check=True)
                    if sp:
                        # drain the quad's accumulators (deprioritized).
                        ctx_hp = tc.high_priority(offset=-60)
                        ctx_hp.__enter__()
                        qs = slice(512 * qc, 512 * (qc + 1))
                        for a in range(2):
                            av = av_of[hq][a]
                            pA = 64 * a
                            oA = (64 * a + 64) % 128
                            if a == 0:
                                nc.scalar.copy(
                                    attn_sb[hq][pA:pA + 64, qs],
                                    av[pA:pA + 64, :])
                                nc.vector.tensor_copy(
                                    rec_bc[hq][pA:pA + 64, qs],
                                    av[oA:oA + 64, :])
                            else:
                                nc.vector.tensor_copy(
                                    attn_sb[hq][pA:pA + 64, qs],
                                    av[pA:pA + 64, :])
                                nc.scalar.copy(
                                    rec_bc[hq][pA:pA + 64, qs],
                                    av[oA:oA + 64, :])
                        ctx_hp.__exit__(None, None, None)

                def view3(t):
                    import concourse.bass as bass
                    return bass.AP(tensor=t.tensor, offset=t.offset,
                                   ap=[t.ap[0], [D, 4], [1, D]])

                def emit_v(i4):
                    # V[k,d] for 4 k-chunks through one score-rotation
                    # tile; one [128,1024] copy-out.
                    vp = scps.tile([128, 1024], F32, name="sc")
                    vpv = view3(vp)
                    for j in range(4):
                        kt_i = i4 * 4 + j
                        for c in range(2):
                            nc.tensor.matmul(
                                vpv[:, j, :],
                                xc_sb[c][:, 128 * kt_i:128 * (kt_i + 1)],
                                wvt_sb[c], start=(c == 0), stop=(c == 1))
                    if i4 % 2 == 0:
                        nc.vector.tensor_copy(
                            v_sb[:, 4 * i4:4 * (i4 + 1), :], vpv)
                    else:
                        nc.scalar.copy(
                            v_sb[:, 4 * i4:4 * (i4 + 1), :], vpv)

                units = [(qc, hq, kc) for qc in range(2) for hq in range(2)
                         for kc in range(KC)]
                NU = len(units)
                # V head, attention prologue, V tail, then the pipeline:
                # scores+exps for unit i+1, AV for unit i.
                for i4 in range(5):
                    emit_v(i4)
                emit_scores(0, 0, 0)
                emit_exps(0, 0, 0)
                for i4 in range(5, 8):
                    emit_v(i4)
                for i in range(NU):
                    if i + 1 < NU:
                        emit_scores(*units[i + 1])
                        emit_exps(*units[i + 1])
                    emit_avs(*units[i])

        # normalize attn_out^T by 1/sumexp. The reciprocal runs on
        # ScalarE as exp(-ln(x)) — both live in the one ACT table set
        # this kernel uses, and this avoids the DVE's iterative-divide.
        for g in range(2):
            nc.scalar.activation(rec_bc[g], rec_bc[g], AF.Ln)
        for g in range(2):
            nc.scalar.activation(rec_bc[g], rec_bc[g], AF.Exp, scale=-1.0)
        nc.vector.tensor_mul(attn_sb[0], attn_sb[0], rec_bc[0])
        nc.gpsimd.tensor_mul(attn_sb[1], attn_sb[1], rec_bc[1])

        # =============== Phase C: Wo+resid + LN1 + transpose ===========
        with tc.tile_pool(name="post", bufs=1) as POST, \
             tc.tile_pool(name="lnp", bufs=6) as lnp, \
             tc.tile_pool(name="x1p", bufs=3) as x1p:
            x1t_sb = [POST.tile([128, NQ], BF16, name=f"x1t{g}")
                      for g in range(2)]
            hdn_sb = POST.tile([128, 8, NQ], BF16, name="hdn_sb")

            def ln_stats(src_psum, i):
                # mean/rstd of src rows; rstd = exp(-0.5 ln(var+eps)).
                stats = lnp.tile([128, 6], F32, name="stats")
                nc.vector.bn_stats(out=stats, in_=src_psum)
                mv = lnp.tile([128, 2], F32, name="mv")
                nc.vector.bn_aggr(out=mv, in_=stats)
                rstd = lnp.tile([128, 1], F32, name="rstd")
                nc.scalar.activation(rstd, mv[:, 1:2], AF.Ln, bias=eps_sb)
                nc.scalar.activation(rstd, rstd, AF.Exp, scale=-0.5)
                return mv, rstd

            def ln_apply(dst, src_psum, mv, rstd, i):
                # dst = (src - mean) * rstd, alternating engines.
                if i % 2 == 0:
                    nc.vector.tensor_scalar(
                        out=dst, in0=src_psum, scalar1=mv[:, 0:1],
                        scalar2=rstd, op0=OP.subtract, op1=OP.mult)
                else:
                    nmr = lnp.tile([128, 1], F32, name="nmr")
                    nc.vector.tensor_scalar(
                        out=nmr, in0=mv[:, 0:1], scalar1=rstd,
                        scalar2=-1.0, op0=OP.mult, op1=OP.mult)
                    nc.scalar.activation(dst, src_psum, AF.Identity,
                                         bias=nmr, scale=rstd)

            with tc.tile_pool(name="ppps", bufs=3, space="PSUM") as ppps, \
                 tc.tile_pool(name="tpps", bufs=4, space="PSUM") as tpps:
                for qt_i in range(QT_TILES):
                    ts = slice(128 * qt_i, 128 * (qt_i + 1))
                    pp = ppps.tile([128, D], F32, name="pp")
                    # pp = attn@Wo + xl@Wres (resid folded into the group)
                    nc.tensor.matmul(pp, attn_sb[0][:, ts], wot_sb[0],
                                     start=True, stop=False)
                    nc.tensor.matmul(pp, attn_sb[1][:, ts], wot_sb[1],
                                     start=False, stop=False)
                    nc.tensor.matmul(pp, xqf_sb[:, ts], wrt_sb,
                                     start=False, stop=True)
                    mv, rstd = ln_stats(pp, qt_i)
                    x1n = x1p.tile([128, D], F32, name="x1n")
                    ln_apply(x1n, pp, mv, rstd, qt_i)
                    # transpose x1n; fold the LN1 g/b affine into the
                    # copy-out (per-partition in the transposed layout).
                    for dc in range(2):
                        tp = tpps.tile([128, 128], F32, name="tp")
                        nc.tensor.transpose(
                            tp, x1n[:, 128 * dc:128 * (dc + 1)], ident)
                        dst = x1t_sb[dc][:, ts]
                        if (qt_i + dc) % 2 == 0:
                            nc.vector.tensor_scalar(
                                out=dst, in0=tp,
                                scalar1=g1_col[:, dc:dc + 1],
                                scalar2=b1_col[:, dc:dc + 1],
                                op0=OP.mult, op1=OP.add)
                        else:
                            nc.scalar.activation(
                                dst, tp, AF.Identity,
                                bias=b1_col[:, dc:dc + 1],
                                scale=g1_col[:, dc:dc + 1])

            # =============== Phase D: FFN + LN2 ===============
            # FFN1 and FFN2 interleave per q-half so the PE stream stays
            # dense: hdn^T[f,q] = relu(sum_d W1T[d,f] x1T[d,q] + bf1),
            # then ffn[q,d] = sum_f hdnT[f,q] W2T[f,d] + bf2 + x1
            # (residual and bias enter as extra accumulating matmuls),
            # x2 = LN2(.).
            import concourse.bass as bass

            def rep4(t):
                return bass.AP(tensor=t.tensor, offset=t.offset,
                               ap=[t.ap[0], [0, 4], t.ap[1]])

            x2n_sb = POST.tile([128, QT_TILES, D], F32, name="x2n_sb")
            x2a_sb = POST.tile([128, QT_TILES, D], F32, name="x2a_sb")
            with tc.tile_pool(name="ffps", bufs=3, space="PSUM") as ffps, \
                 tc.tile_pool(name="f2ps", bufs=4, space="PSUM") as f2ps:
                for qc in range(2):
                    qs = slice(512 * qc, 512 * (qc + 1))
                    for fc in range(8):
                        hp_ = ffps.tile([128, 512], F32, name="hp_")
                        for dc in range(2):
                            nc.tensor.matmul(
                                hp_, w1t_sb[dc][:, 128 * fc:128 * (fc + 1)],
                                x1t_sb[dc][:, qs],
                                start=(dc == 0), stop=(dc == 1))
                        if (fc + qc) % 2 == 0:
                            nc.scalar.activation(
                                hdn_sb[:, fc, qs], hp_, AF.Relu,
                                bias=bf1_col[:, fc:fc + 1])
                        else:
                            nc.vector.tensor_scalar(
                                out=hdn_sb[:, fc, qs], in0=hp_,
                                scalar1=bf1_col[:, fc:fc + 1], scalar2=0.0,
                                op0=OP.add, op1=OP.max)
                    for qt_i in range(4 * qc, 4 * qc + 4):
                        ts = slice(128 * qt_i, 128 * (qt_i + 1))
                        fp = f2ps.tile([128, D], F32, name="fp")
                        for fc in range(8):
                            nc.tensor.matmul(fp, hdn_sb[:, fc, ts],
                                             w2t_sb[:, fc, :],
                                             start=(fc == 0), stop=False)
                        nc.tensor.matmul(fp, ones_row, bf2_row,
                                         start=False, stop=False)
                        for dc in range(2):
                            nc.tensor.matmul(
                                fp, x1t_sb[dc][:, ts],
                                i2_bf[:, 256 * dc:256 * (dc + 1)],
                                start=False, stop=(dc == 1))
                        mv, rstd = ln_stats(fp, qt_i)
                        ln_apply(x2n_sb[:, qt_i, :], fp, mv, rstd, qt_i + 1)
                    # final affine *g2+b2 for this half (DVE / GpSimd),
                    # then store
                    hs = slice(4 * qc, 4 * (qc + 1))
                    eng = nc.vector if qc == 0 else nc.gpsimd
                    eng.tensor_mul(x2a_sb[:, hs, :], x2n_sb[:, hs, :],
                                   rep4(g2_bc))
                    eng.tensor_add(x2a_sb[:, hs, :], x2a_sb[:, hs, :],
                                   rep4(b2_bc))
                    for qt_i in range(4 * qc, 4 * qc + 4):
                        ts = slice(128 * qt_i, 128 * (qt_i + 1))
                        nc.sync.dma_start(out[ts, :], x2a_sb[:, qt_i, :])

    nc.compile()
    return nc


def _get_nc():
    global _built
    if _built is None:
        _built = _build()
    return _built


def _make_in_maps(inputs):
    import ml_dtypes
    f32 = np.float32
    bf16 = ml_dtypes.bfloat16
    F_lidar = np.ascontiguousarray(inputs["F_lidar"], dtype=f32)
    F_cam = np.ascontiguousarray(inputs["F_cam"], dtype=f32)
    common = {
        "wkt": np.ascontiguousarray(np.asarray(inputs["Wk"]).T.astype(bf16)),
        "wvt": np.ascontiguousarray(np.asarray(inputs["Wv"]).T.astype(bf16)),
        "wqt": np.ascontiguousarray(np.asarray(inputs["Wq"]).T.astype(bf16)),
        "wrt": np.ascontiguousarray(inputs["Wres"].T, f32),
        "wot": np.ascontiguousarray(np.asarray(inputs["Wo"]).T.astype(bf16)),
        "w1t": np.ascontiguousarray(np.asarray(inputs["W1"]).T.astype(bf16)),
        "w2t": np.ascontiguousarray(np.asarray(inputs["W2"]).T.astype(bf16)),
        "g1": np.asarray(inputs["g1"], f32), "b1": np.asarray(inputs["b1"], f32),
        "g2": np.asarray(inputs["g2"], f32), "b2": np.asarray(inputs["b2"], f32),
        "bf1": np.asarray(inputs["bf1"], f32),
        "bf2": np.asarray(inputs["bf2"]).astype(bf16),
    }
    in_maps = []
    for c in range(N_CORES):
        b, s = c // CORES_PER_B, (c % CORES_PER_B) * NQ
        m = dict(common)
        xq_f = np.ascontiguousarray(
            F_lidar[b].reshape(C1, N_TOK)[:, s:s + NQ])
        m["xq"] = xq_f.astype(bf16)
        m["xqf"] = xq_f
        m["xc"] = np.ascontiguousarray(
            F_cam[b].reshape(C2, N_TOK)).astype(bf16)
        in_maps.append(m)
    return in_maps


def kernel(**inputs):
    from concourse.bass_utils import run_bass_kernel_spmd

    nc = _get_nc()
    in_maps = _make_in_maps(inputs)
    res = run_bass_kernel_spmd(nc, in_maps, list(range(N_CORES)))
    out = np.empty((B, D, N_TOK), dtype=np.float32)
    for c in range(N_CORES):
        b, s = c // CORES_PER_B, (c % CORES_PER_B) * NQ
        out[b, :, s:s + NQ] = res.results[c]["out"].T
    return out.reshape(B, D, H, W)


# revision 30
# speedup vs baseline: 1.0394x; 1.0394x over previous
"""Trainium2 Bass kernel for nn_CrossAttentionFusion.

Problem (hardcoded shapes): B=2, C1=64, C2=256, D=256, NH=8, HD=32, H=W=64,
n = H*W = 4096 tokens per batch image.

    xl = F_lidar tokens (B, n, C1); xc = F_cam tokens (B, n, C2)
    Q = xl@Wq^T, K = xc@Wk^T, V = xc@Wv^T  (per-head HD=32)
    attn = softmax(QK^T/sqrt(HD)); out = attn@V
    x = LN1(xl@Wres^T + out@Wo^T); x = LN2(x + FFN(x)); return (B, D, H, W)

Sharding: 8 cores, zero collectives. Core i handles batch b=i//4 and the
1024-token q-slice (i%4). K/V for the whole image are recomputed per core.

v3 changes (vs the 343us v2):
  * Startup: input DMAs fan out over four engine queues (sync/tensor/
    vector/gpsimd) instead of serializing ~22us on the sync queue.
  * One ACT table set for the whole kernel (natural_log_exp_and_others):
    LN rstd is exp(-0.5*ln(var+eps)) instead of Sqrt + DVE reciprocal, so
    no mid-kernel ACT_TABLE_LOAD thrash (6 x 1.3us on the v2 trace).
  * resid = xl@Wres is folded into the Wo PSUM accumulation in phase C
    (3 accumulating matmuls per q-tile); the separate phase-A resid pass,
    its SBUF buffer, and its DVE copies are gone.
  * Phase C/D rebalanced: LN apply and transpose-copyouts alternate
    ScalarE(Identity scale/bias per-partition)/DVE, LN1 affine folded into
    the transpose copy-out (g1/b1 are per-partition in the transposed
    layout), FFN1 relu+bias alternates ScalarE/DVE, bf2 enters FFN2 as a
    1-row ones matmul, LN2 residual (x1) re-enters via identity-rhs
    matmuls accumulating into the FFN2 PSUM, final g2/b2 affine split
    DVE/GpSimd.
  * KT/QT/V PSUM->SBUF copy-outs alternate ScalarE/DVE.

Attention core (unchanged from v2): bf16 QK^T with 2-head row packing;
exp split ScalarE Exp (qc=0) / DVE Schraudolph int16 (qc=1); AV + ones-
denominator column-packed 4-wide; 3-deep PSUM score rotation.
"""

import numpy as np

B, C1, C2, D, NH, H, W = 2, 64, 256, 256, 8, 64, 64
HD = D // NH                 # 32
N_TOK = H * W                # 4096 tokens per image
N_CORES = 8
CORES_PER_B = N_CORES // B   # 4
NQ = N_TOK // CORES_PER_B    # 1024 q tokens per core
EPS = 1e-5
SCALE = HD ** -0.5
KC = N_TOK // 128            # 32 k-chunks
QT_TILES = NQ // 128         # 8 q-tiles of 128
F1 = 4 * D                   # 1024 FFN hidden

# Schraudolph-style exp for bf16-bit-pattern in int16 (trunc semantics):
#   bf16_bits(exp(s*SCALE)) ~= int16(EXP_A*s + EXP_B)
EXP_A = 184.6649652337873 * SCALE
EXP_B = 16250.89

_built = None


def _build():
    from contextlib import ExitStack

    import concourse.mybir as mybir
    import concourse.tile as tile
    from concourse import bacc
    from concourse.masks import make_identity

    F32 = mybir.dt.float32
    BF16 = mybir.dt.bfloat16
    I16 = mybir.dt.int16
    I8 = mybir.dt.int8
    FP8 = mybir.dt.float8e4
    FP8E5 = mybir.dt.float8e5
    AF = mybir.ActivationFunctionType
    OP = mybir.AluOpType

    import bass_rust as _bass_rust
    from concourse.hw_specs import get_activation_tables

    class OneSetBacc(bacc.Bacc):
        # Force every ACT table load to natural_log_exp_and_others (it
        # contains exp/ln/identity/copy/relu — everything this kernel
        # uses), so the kernel pays exactly one ACT_TABLE_LOAD instead
        # of thrashing between per-function home sets.
        def insert_act_table_loads(self):
            has_activation = any(
                isinstance(i, mybir.InstActivation)
                for b in self.main_func.blocks
                for i in b.instructions
            )
            if not has_activation:
                return
            keep = "natural_log_exp_and_others"
            tables = [(name, (fns if name == keep else set()))
                      for name, fns in
                      get_activation_tables(self.m.arch).items()]
            _bass_rust.insert_act_table_loads(self, tables)

    nc = OneSetBacc(trn_type="TRN2", target_bir_lowering=False, debug=False,
                    num_devices=N_CORES)

    # ---- DRAM I/O ----
    xq = nc.dram_tensor("xq", [C1, NQ], BF16, kind="ExternalInput").ap()
    xqf = nc.dram_tensor("xqf", [C1, NQ], F32, kind="ExternalInput").ap()
    xc = nc.dram_tensor("xc", [C2, N_TOK], BF16, kind="ExternalInput").ap()
    wkt = nc.dram_tensor("wkt", [C2, D], BF16, kind="ExternalInput").ap()
    wvt = nc.dram_tensor("wvt", [C2, D], BF16, kind="ExternalInput").ap()
    wqt = nc.dram_tensor("wqt", [C1, D], BF16, kind="ExternalInput").ap()
    wrt = nc.dram_tensor("wrt", [C1, D], F32, kind="ExternalInput").ap()
    wot = nc.dram_tensor("wot", [D, D], BF16, kind="ExternalInput").ap()
    w1t = nc.dram_tensor("w1t", [D, F1], BF16, kind="ExternalInput").ap()
    w2t = nc.dram_tensor("w2t", [F1, D], BF16, kind="ExternalInput").ap()
    g1 = nc.dram_tensor("g1", [D], F32, kind="ExternalInput").ap()
    b1 = nc.dram_tensor("b1", [D], F32, kind="ExternalInput").ap()
    g2 = nc.dram_tensor("g2", [D], F32, kind="ExternalInput").ap()
    b2 = nc.dram_tensor("b2", [D], F32, kind="ExternalInput").ap()
    bf1 = nc.dram_tensor("bf1", [F1], F32, kind="ExternalInput").ap()
    bf2 = nc.dram_tensor("bf2", [D], BF16, kind="ExternalInput").ap()
    out = nc.dram_tensor("out", [NQ, D], F32, kind="ExternalOutput").ap()

    with tile.TileContext(nc) as tc, ExitStack() as ctx:
        # ---- persistent SBUF ----
        P = ctx.enter_context(tc.tile_pool(name="persist", bufs=1))

        xq_sb = P.tile([C1, NQ], BF16, name="xq_sb")
        wkt_sb = [P.tile([128, D], BF16, name=f"wkt{c}") for c in range(2)]
        wvt_sb = [P.tile([128, D], BF16, name=f"wvt{c}") for c in range(2)]
        wqt_sb = P.tile([C1, D], BF16, name="wqt_sb")
        wrt_sb = P.tile([128, D], F32, name="wrt_sb")
        xqf_sb = P.tile([128, NQ], F32, name="xqf_sb")
        wot_sb = [P.tile([128, D], BF16, name=f"wot{c}") for c in range(2)]
        kt_sb = [P.tile([128, N_TOK], BF16, name=f"kt{g}") for g in range(2)]
        v_sb = P.tile([128, KC, D], BF16, name="v_sb")
        qt_sb = [P.tile([128, NQ], BF16, name=f"qt{g}") for g in range(2)]
        attn_sb = [P.tile([128, NQ], BF16, name=f"attn{g}") for g in range(2)]
        rec_bc = [P.tile([128, NQ], F32, name=f"recbc{g}") for g in range(2)]
        ones_bf = P.tile([128, HD], BF16, name="ones_bf")
        ident = P.tile([128, 128], F32, name="ident")
        i2_bf = P.tile([128, 512], BF16, name="i2_bf")
        eps_sb = P.tile([128, 1], F32, name="eps_sb")
        g1_col = P.tile([128, 2], F32, name="g1_col")
        b1_col = P.tile([128, 2], F32, name="b1_col")
        g2_bc = P.tile([128, D], F32, name="g2_bc")
        b2_bc = P.tile([128, D], F32, name="b2_bc")
        bf1_col = P.tile([128, 8], F32, name="bf1_col")
        ones_row = P.tile([1, 128], BF16, name="ones_row")
        bf2_row = P.tile([1, D], BF16, name="bf2_row")

        ones_f32 = P.tile([128, HD], F32, name="ones_f32")
        nc.vector.memset(ones_f32, 1.0)
        nc.vector.memset(xqf_sb[C1:128, :], 0.0)
        nc.vector.memset(wrt_sb[C1:128, :], 0.0)
        nc.vector.tensor_copy(ones_bf, ones_f32)
        nc.vector.memset(ones_row, 1.0)
        nc.vector.memset(eps_sb, EPS)
        make_identity(nc, ident)
        nc.vector.memset(i2_bf, 0.0)
        nc.vector.tensor_copy(i2_bf[:, 0:128], ident)
        nc.vector.tensor_copy(i2_bf[:, 384:512], ident)

        def bcast_row(dst, src_ap, eng=None):
            # (n,) dram -> (128, n) sbuf, replicated on all partitions
            import concourse.bass as bass
            src = bass.AP(tensor=src_ap.tensor, offset=src_ap.offset,
                          ap=[[0, 128]] + src_ap.ap)
            (eng or nc.sync).dma_start(dst, src)

        # --- startup DMAs fanned out across the three DMA-capable
        # queues (sync/scalar/gpsimd) instead of serializing on sync ---
        # sync: the K-projection critical path (wkt, then xc half 0).
        for c in range(2):
            nc.sync.dma_start(wkt_sb[c], wkt[128 * c:128 * (c + 1), :])
        # scalar queue: Q-side + xc half 1 + V/resid weights (all done
        # well before ScalarE's first PSUM copy-out).
        nc.scalar.dma_start(wqt_sb, wqt)
        nc.scalar.dma_start(xq_sb, xq)
        # gpsimd queue: LN constants + phase C/D weights.
        nc.gpsimd.dma_start(g1_col, g1.rearrange("(a p) -> p a", p=128))
        nc.gpsimd.dma_start(b1_col, b1.rearrange("(a p) -> p a", p=128))
        bcast_row(g2_bc, g2, nc.gpsimd)
        bcast_row(b2_bc, b2, nc.gpsimd)
        nc.gpsimd.dma_start(bf1_col, bf1.rearrange("(a p) -> p a", p=128))
        nc.gpsimd.dma_start(bf2_row, bf2.rearrange("(o d) -> o d", o=1))

        # =============== Phase A: projections (bf16) ===============
        # KT/QT first in their own PSUM pool. V then runs THROUGH the
        # attention score-buffer rotation, so the attention prologue
        # (first scores + exps) interleaves mid-V and the PE never sees
        # an idle window at the phase transition.
        with tc.tile_pool(name="xc_pool", bufs=1) as XP:
            xc_sb = [XP.tile([128, N_TOK], BF16, name=f"xc{c}")
                     for c in range(2)]
            for ch in range(4):
                cs = slice(1024 * ch, 1024 * (ch + 1))
                nc.sync.dma_start(xc_sb[0][:, cs], xc[0:128, cs])
                nc.scalar.dma_start(xc_sb[1][:, cs], xc[128:256, cs])
            for c in range(2):
                nc.scalar.dma_start(wvt_sb[c], wvt[128 * c:128 * (c + 1), :])
            nc.scalar.dma_start(xqf_sb[0:C1, :], xqf)
            nc.scalar.dma_start(wrt_sb[0:C1, :], wrt)
            for c in range(2):
                nc.scalar.dma_start(wot_sb[c], wot[128 * c:128 * (c + 1), :])
            # phase D weights on the gpsimd queue, behind the LN consts.
            w1t_sb = [P.tile([128, F1], BF16, name=f"w1t{c}")
                      for c in range(2)]
            for c in range(2):
                nc.gpsimd.dma_start(w1t_sb[c], w1t[128 * c:128 * (c + 1), :])
            w2t_sb = P.tile([128, 8, D], BF16, name="w2t_sb")
            nc.gpsimd.dma_start(
                w2t_sb, w2t.rearrange("(a p) d -> p a d", p=128))

            with tc.tile_pool(name="psA", bufs=4, space="PSUM") as psA:
                # KT[d,k] = sum_c WkT[c,d] * xcT[c,k]
                for g in range(2):
                    for ks in range(8):
                        kp = psA.tile([128, 512], F32, name="kp")
                        for c in range(2):
                            nc.tensor.matmul(
                                kp, wkt_sb[c][:, 128 * g:128 * (g + 1)],
                                xc_sb[c][:, 512 * ks:512 * (ks + 1)],
                                start=(c == 0), stop=(c == 1))
                        dst = kt_sb[g][:, 512 * ks:512 * (ks + 1)]
                        if ks % 2 == 0:
                            nc.scalar.copy(dst, kp)
                        else:
                            nc.vector.tensor_copy(dst, kp)
                # QT[d,q] = sum_c WqT[c,d] * xqT[c,q]
                for g in range(2):
                    for qs in range(NQ // 512):
                        qp = psA.tile([128, 512], F32, name="kp")
                        nc.tensor.matmul(
                            qp, wqt_sb[:, 128 * g:128 * (g + 1)],
                            xq_sb[:, 512 * qs:512 * (qs + 1)],
                            start=True, stop=True)
                        dst = qt_sb[g][:, 512 * qs:512 * (qs + 1)]
                        if qs % 2 == 0:
                            nc.scalar.copy(dst, qp)
                        else:
                            nc.vector.tensor_copy(dst, qp)

            # ===== Phase B: V tail + attention =====
            # Attention runs in two q-half passes. Per (head-quad, kc)
            # unit one span of FOUR row-packed score matmuls (rows
            # 0/32/64/96, N=512) writes two [128,1024] PSUM tiles -- four
            # distinct banks, so the concurrent writes are legal. The e
            # tiles keep the baseline [hA 512q | hB 512q] layout, so AV +
            # ones-denominator reuse the proven column-packed 4-wide
            # structure on [128,512] accumulators (1 bank per head-pair).
            # exp alternates ScalarE Exp / DVE int16 Schraudolph per tile.
            with tc.tile_pool(name="scps", bufs=3, space="PSUM") as scps, \
                 tc.tile_pool(name="avps", bufs=1, space="PSUM") as avps, \
                 tc.tile_pool(name="epool", bufs=10) as epool:
                scs, es, av_of = {}, {}, {}

                def emit_scores(qc, hq, kc):
                    ks = slice(128 * kc, 128 * (kc + 1))
                    qs = slice(512 * qc, 512 * (qc + 1))
                    tiles = []
                    for a in range(2):
                        sc = scps.tile([128, 1024], F32, name="sc")
                        scs[(qc, hq, kc, a)] = sc
                        tiles.append(sc)
                    for m in range(4):
                        nc.tensor.matmul(
                            tiles[m // 2][:, 512 * (m % 2):512 * (m % 2 + 1)],
                            kt_sb[hq][32 * m:32 * m + 32, ks],
                            qt_sb[hq][32 * m:32 * m + 32, qs],
                            start=True, stop=True,
                            tile_position=(32 * m, 0))

                def emit_exps(qc, hq, kc):
                    for a in range(2):
                        sc = scs.pop((qc, hq, kc, a))
                        e = epool.tile([128, 1024], BF16, name="e")
                        es[(qc, hq, kc, a)] = e
                        if a == 0:
                            nc.scalar.activation(e, sc, AF.Exp, scale=SCALE)
                        else:
                            nc.vector.tensor_scalar(
                                out=e.bitcast(I16), in0=sc,
                                scalar1=EXP_A, scalar2=EXP_B,
                                op0=OP.mult, op1=OP.add)

                def emit_avs(qc, hq, kc):
                    if kc == 0:
                        av_of[hq] = [
                            avps.tile([128, 512], F32, name=f"av{a}")
                            for a in range(2)]
                    st, sp = (kc == 0), (kc == KC - 1)
                    for a in range(2):
                        av = av_of[hq][a]
                        hA, hB = 4 * hq + 2 * a, 4 * hq + 2 * a + 1
                        pA, pB = 64 * a, 64 * a + 32
                        oA, oB = (64 * a + 64) % 128, (64 * a + 96) % 128
                        e = es.pop((qc, hq, kc, a))
                        nc.tensor.matmul(
                            av[pA:pA + 32, :],
                            v_sb[:, kc, HD * hA:HD * hA + HD],
                            e[:, 0:512], start=st, stop=sp,
                            tile_position=(0, pA), skip_group_check=True)
                        nc.tensor.matmul(
                            av[pB:pB + 32, :],
                            v_sb[:, kc, HD * hB:HD * hB + HD],
                            e[:, 512:1024], start=st, stop=sp,
                            tile_position=(0, pB), skip_group_check=True)
                        nc.tensor.matmul(
                            av[oA:oA + 32, :], ones_bf, e[:, 0:512],
                            start=st, stop=sp, tile_position=(0, oA),
                            skip_group_check=True)
                        nc.tensor.matmul(
                            av[oB:oB + 32, :], ones_bf, e[:, 512:1024],
                            start=st, stop=sp, tile_position=(0, oB),
                            skip_group_check=True)
                    if sp:
                        # drain the quad's accumulators (deprioritized).
                        ctx_hp = tc.high_priority(offset=-60)
                        ctx_hp.__enter__()
                        qs = slice(512 * qc, 512 * (qc + 1))
                        for a in range(2):
                            av = av_of[hq][a]
                            pA = 64 * a
                            oA = (64 * a + 64) % 128
                            if a == 0:
                                nc.scalar.copy(
                                    attn_sb[hq][pA:pA + 64, qs],
                                    av[pA:pA + 64, :])
                                nc.vector.tensor_copy(
                                    rec_bc[hq][pA:pA + 64, qs],
                                    av[oA:oA + 64, :])
                            else:
                                nc.vector.tensor_copy(
                                    attn_sb[hq][pA:pA + 64, qs],
                                    av[pA:pA + 64, :])
                                nc.scalar.copy(
                                    rec_bc[hq][pA:pA + 64, qs],
                                    av[oA:oA + 64, :])
                        ctx_hp.__exit__(None, None, None)

                def view3(t):
                    import concourse.bass as bass
                    return bass.AP(tensor=t.tensor, offset=t.offset,
                                   ap=[t.ap[0], [D, 4], [1, D]])

                def emit_v(i4):
                    # V[k,d] for 4 k-chunks through one score-rotation
                    # tile; one [128,1024] copy-out.
                    vp = scps.tile([128, 1024], F32, name="sc")
                    vpv = view3(vp)
                    for j in range(4):
                        kt_i = i4 * 4 + j
                        for c in range(2):
                            nc.tensor.matmul(
                                vpv[:, j, :],
                                xc_sb[c][:, 128 * kt_i:128 * (kt_i + 1)],
                                wvt_sb[c], start=(c == 0), stop=(c == 1))
                    if i4 % 2 == 0:
                        nc.vector.tensor_copy(
                            v_sb[:, 4 * i4:4 * (i4 + 1), :], vpv)
                    else:
                        nc.scalar.copy(
                            v_sb[:, 4 * i4:4 * (i4 + 1), :], vpv)

                units = [(qc, hq, kc) for qc in range(2) for hq in range(2)
                         for kc in range(KC)]
                NU = len(units)
                # V head, attention prologue, V tail, then the pipeline:
                # scores+exps for unit i+1, AV for unit i.
                for i4 in range(5):
                    emit_v(i4)
                emit_scores(0, 0, 0)
                emit_exps(0, 0, 0)
                for i4 in range(5, 8):
                    emit_v(i4)
                for i in range(NU):
                    if i + 1 < NU:
                        emit_scores(*units[i + 1])
                        emit_exps(*units[i + 1])
                    emit_avs(*units[i])

        # normalize attn_out^T by 1/sumexp. The reciprocal runs on
        # ScalarE as exp(-ln(x)) — both live in the one ACT table set
        # this kernel uses, and this avoids the DVE's iterative-divide.
        for g in range(2):
            nc.scalar.activation(rec_bc[g], rec_bc[g], AF.Ln)
        for g in range(2):
            nc.scalar.activation(rec_bc[g], rec_bc[g], AF.Exp, scale=-1.0)
        nc.vector.tensor_mul(attn_sb[0], attn_sb[0], rec_bc[0])
        nc.gpsimd.tensor_mul(attn_sb[1], attn_sb[1], rec_bc[1])

        # =============== Phase C: Wo+resid + LN1 + transpose ===========
        with tc.tile_pool(name="post", bufs=1) as POST, \
             tc.tile_pool(name="lnp", bufs=6) as lnp, \
             tc.tile_pool(name="x1p", bufs=3) as x1p:
            x1t_sb = [POST.tile([128, NQ], BF16, name=f"x1t{g}")
                      for g in range(2)]
            hdn_sb = POST.tile([128, 8, NQ], BF16, name="hdn_sb")

            def ln_stats(src_psum, i):
                # mean/rstd of src rows; rstd = exp(-0.5 ln(var+eps)).
                stats = lnp.tile([128, 6], F32, name="stats")
                nc.vector.bn_stats(out=stats, in_=src_psum)
                mv = lnp.tile([128, 2], F32, name="mv")
                nc.vector.bn_aggr(out=mv, in_=stats)
                rstd = lnp.tile([128, 1], F32, name="rstd")
                nc.scalar.activation(rstd, mv[:, 1:2], AF.Ln, bias=eps_sb)
                nc.scalar.activation(rstd, rstd, AF.Exp, scale=-0.5)
                return mv, rstd

            def ln_apply(dst, src_psum, mv, rstd, i):
                # dst = (src - mean) * rstd, alternating engines.
                if i % 2 == 0:
                    nc.vector.tensor_scalar(
                        out=dst, in0=src_psum, scalar1=mv[:, 0:1],
                        scalar2=rstd, op0=OP.subtract, op1=OP.mult)
                else:
                    nmr = lnp.tile([128, 1], F32, name="nmr")
                    nc.vector.tensor_scalar(
                        out=nmr, in0=mv[:, 0:1], scalar1=rstd,
                        scalar2=-1.0, op0=OP.mult, op1=OP.mult)
                    nc.scalar.activation(dst, src_psum, AF.Identity,
                                         bias=nmr, scale=rstd)

            with tc.tile_pool(name="ppps", bufs=3, space="PSUM") as ppps, \
                 tc.tile_pool(name="tpps", bufs=4, space="PSUM") as tpps:
                for qt_i in range(QT_TILES):
                    ts = slice(128 * qt_i, 128 * (qt_i + 1))
                    pp = ppps.tile([128, D], F32, name="pp")
                    # pp = attn@Wo + xl@Wres (resid folded into the group)
                    nc.tensor.matmul(pp, attn_sb[0][:, ts], wot_sb[0],
                                     start=True, stop=False)
                    nc.tensor.matmul(pp, attn_sb[1][:, ts], wot_sb[1],
                                     start=False, stop=False)
                    nc.tensor.matmul(pp, xqf_sb[:, ts], wrt_sb,
                                     start=False, stop=True)
                    mv, rstd = ln_stats(pp, qt_i)
                    x1n = x1p.tile([128, D], F32, name="x1n")
                    ln_apply(x1n, pp, mv, rstd, qt_i)
                    # transpose x1n; fold the LN1 g/b affine into the
                    # copy-out (per-partition in the transposed layout).
                    for dc in range(2):
                        tp = tpps.tile([128, 128], F32, name="tp")
                        nc.tensor.transpose(
                            tp, x1n[:, 128 * dc:128 * (dc + 1)], ident)
                        dst = x1t_sb[dc][:, ts]
                        if (qt_i + dc) % 2 == 0:
                            nc.vector.tensor_scalar(
                                out=dst, in0=tp,
                                scalar1=g1_col[:, dc:dc + 1],
                                scalar2=b1_col[:, dc:dc + 1],
                                op0=OP.mult, op1=OP.add)
                        else:
                            nc.scalar.activation(
                                dst, tp, AF.Identity,
                                bias=b1_col[:, dc:dc + 1],
                                scale=g1_col[:, dc:dc + 1])

            # =============== Phase D: FFN + LN2 ===============
            # FFN1 and FFN2 interleave per q-half so the PE stream stays
            # dense: hdn^T[f,q] = relu(sum_d W1T[d,f] x1T[d,q] + bf1),
            # then ffn[q,d] = sum_f hdnT[f,q] W2T[f,d] + bf2 + x1
            # (residual and bias enter as extra accumulating matmuls),
            # x2 = LN2(.).
            import concourse.bass as bass

            def rep4(t):
                return bass.AP(tensor=t.tensor, offset=t.offset,
                               ap=[t.ap[0], [0, 4], t.ap[1]])

            x2n_sb = POST.tile([128, QT_TILES, D], F32, name="x2n_sb")
            x2a_sb = POST.tile([128, QT_TILES, D], F32, name="x2a_sb")
            with tc.tile_pool(name="ffps", bufs=3, space="PSUM") as ffps, \
                 tc.tile_pool(name="f2ps", bufs=4, space="PSUM") as f2ps:
                for qc in range(2):
                    qs = slice(512 * qc, 512 * (qc + 1))
                    for fc in range(8):
                        hp_ = ffps.tile([128, 512], F32, name="hp_")
                        for dc in range(2):
                            nc.tensor.matmul(
                                hp_, w1t_sb[dc][:, 128 * fc:128 * (fc + 1)],
                                x1t_sb[dc][:, qs],
                                start=(dc == 0), stop=(dc == 1))
                        if (fc + qc) % 2 == 0:
                            nc.scalar.activation(
                                hdn_sb[:, fc, qs], hp_, AF.Relu,
                                bias=bf1_col[:, fc:fc + 1])
                        else:
                            nc.vector.tensor_scalar(
                                out=hdn_sb[:, fc, qs], in0=hp_,
                                scalar1=bf1_col[:, fc:fc + 1], scalar2=0.0,
                                op0=OP.add, op1=OP.max)
                    for qt_i in range(4 * qc, 4 * qc + 4):
                        ts = slice(128 * qt_i, 128 * (qt_i + 1))
                        fp = f2ps.tile([128, D], F32, name="fp")
                        for fc in range(8):
                            nc.tensor.matmul(fp, hdn_sb[:, fc, ts],
                                             w2t_sb[:, fc, :],
                                             start=(fc == 0), stop=False)
                        nc.tensor.matmul(fp, ones_row, bf2_row,
                                         start=False, stop=False)
                        for dc in range(2):
                            nc.tensor.matmul(
                                fp, x1t_sb[dc][:, ts],
                                i2_bf[:, 256 * dc:256 * (dc + 1)],
                                start=False, stop=(dc == 1))
                        mv, rstd = ln_stats(fp, qt_i)
                        ln_apply(x2n_sb[:, qt_i, :], fp, mv, rstd, qt_i + 1)
                    # final affine *g2+b2 for this half (DVE / GpSimd),
                    # then store
                    hs = slice(4 * qc, 4 * (qc + 1))
                    eng = nc.vector if qc == 0 else nc.gpsimd
                    eng.tensor_mul(x2a_sb[:, hs, :], x2n_sb[:, hs, :],
                                   rep4(g2_bc))
                    eng.tensor_add(x2a_sb[:, hs, :], x2a_sb[:, hs, :],
                                   rep4(b2_bc))
                    for qt_i in range(4 * qc, 4 * qc + 4):
                        ts = slice(128 * qt_i, 128 * (qt_i + 1))
                        nc.sync.dma_start(out[ts, :], x2a_sb[:, qt_i, :])

    nc.compile()
    return nc


def _get_nc():
    global _built
    if _built is None:
        _built = _build()
    return _built


def _make_in_maps(inputs):
    import ml_dtypes
    f32 = np.float32
    bf16 = ml_dtypes.bfloat16
    F_lidar = np.ascontiguousarray(inputs["F_lidar"], dtype=f32)
    F_cam = np.ascontiguousarray(inputs["F_cam"], dtype=f32)
    common = {
        "wkt": np.ascontiguousarray(np.asarray(inputs["Wk"]).T.astype(bf16)),
        "wvt": np.ascontiguousarray(np.asarray(inputs["Wv"]).T.astype(bf16)),
        "wqt": np.ascontiguousarray(np.asarray(inputs["Wq"]).T.astype(bf16)),
        "wrt": np.ascontiguousarray(inputs["Wres"].T, f32),
        "wot": np.ascontiguousarray(np.asarray(inputs["Wo"]).T.astype(bf16)),
        "w1t": np.ascontiguousarray(np.asarray(inputs["W1"]).T.astype(bf16)),
        "w2t": np.ascontiguousarray(np.asarray(inputs["W2"]).T.astype(bf16)),
        "g1": np.asarray(inputs["g1"], f32), "b1": np.asarray(inputs["b1"], f32),
        "g2": np.asarray(inputs["g2"], f32), "b2": np.asarray(inputs["b2"], f32),
        "bf1": np.asarray(inputs["bf1"], f32),
        "bf2": np.asarray(inputs["bf2"]).astype(bf16),
    }
    in_maps = []
    for c in range(N_CORES):
        b, s = c // CORES_PER_B, (c % CORES_PER_B) * NQ
        m = dict(common)
        xq_f = np.ascontiguousarray(
            F_lidar[b].reshape(C1, N_TOK)[:, s:s + NQ])
        m["xq"] = xq_f.astype(bf16)
        m["xqf"] = xq_f
        m["xc"] = np.ascontiguousarray(
            F_cam[b].reshape(C2, N_TOK)).astype(bf16)
        in_maps.append(m)
    return in_maps


def kernel(**inputs):
    from concourse.bass_utils import run_bass_kernel_spmd

    nc = _get_nc()
    in_maps = _make_in_maps(inputs)
    res = run_bass_kernel_spmd(nc, in_maps, list(range(N_CORES)))
    out = np.empty((B, D, N_TOK), dtype=np.float32)
    for c in range(N_CORES):
        b, s = c // CORES_PER_B, (c % CORES_PER_B) * NQ
        out[b, :, s:s + NQ] = res.results[c]["out"].T
    return out.reshape(B, D, H, W)
